# revision 45
# baseline (speedup 1.0000x reference)
"""DeeperGCN forward on 8 Trainium2 NeuronCores (Bass/Tile) — v3.

Strategy (dst-node sharding, batched SWDGE gathers):
- 6250 nodes/core in 49 fixed blocks of 128 (last 106). Per layer each core
  computes its nodes' [P2|P1] = [msg*exp(t*msg) | exp(t*msg)] rows (f16,
  256B), split into group A (blocks 0..24) and B (25..48); AllGather builds
  two replicated tables tabA [25600,128] / tabB [24576,128] — each small
  enough for int16 dma_gather row indices.
- Edge phase: edges live at their dst core, grouped by (src-group, chunk of
  4 dst blocks). One or two dma_gather calls per (group, chunk) fetch ~36
  tiles of 128 src rows per SWDGE instruction (amortizing the ~1us fixed
  descriptor-gen cost). Aggregation per dst block is one-hot matmuls
  accumulated into a 4-block-wide PSUM bank; the num/den softmax epilogue
  (max/recip/mult) runs once per bank instead of once per block.
- One-hot tiles for a whole chunk are built in a single IS_EQ.
- Node phase runs layer-wide per group; identity-valued parameters
  (LN gamma=1/beta=0, zero biases — checked on host) are folded out, the
  final BN scale is folded into fin_W1. Relu/eps/square run on the Scalar
  engine to keep DVE short.
"""

import json
import os
import sys
import types

import numpy as np

sys.path.insert(0, "/opt/trn_rl_repo")

# ---------------------------------------------------------------------------
# Workaround: this walrus build supports only ONE semaphore wait per
# instruction; Tile attaches several. Split extras onto NoOp instructions
# at BIR-JSON serialization time.
# ---------------------------------------------------------------------------
_PATCHED = False


def _install_bir_patch():
    global _PATCHED
    if _PATCHED:
        return
    _PATCHED = True
    import concourse.bass as bass

    orig = bass.Bass.to_json_bytes

    def patched(self):
        data = json.loads(orig(self).decode())
        ctr = 0
        for fn in data.get("functions", []):
            for bb in fn.get("blocks", []):
                new_insts = []
                for inst in bb.get("instructions", []):
                    si = inst.get("sync_info")
                    waits = (si or {}).get("on_wait") or []
                    if len(waits) > 1:
                        for w in waits[:-1]:
                            ctr += 1
                            nop = {
                                "engine": inst["engine"],
                                "ins": [],
                                "outs": [],
                                "name": f"{inst['name']}-sw{ctr}",
                                "opcode": "NoOp",
                                "sync_info": {"on_update": [], "on_wait": [w]},
                            }
                            if "debug" in inst:
                                nop["debug"] = inst["debug"]
                            new_insts.append(nop)
                        si["on_wait"] = [waits[-1]]
                    new_insts.append(inst)
                bb["instructions"] = new_insts
        return json.dumps(data).encode()

    bass.Bass.to_json_bytes = patched


def _install_trace_hook():
    """Optional: register the NTFF profiling hook (for test.py timing)."""
    import antenv

    if "antenv.axon_hooks" in sys.modules:
        return
    _m = types.ModuleType("antenv.axon_hooks")
    _m._hook = None
    _m.set_axon_ntff_profile_hook = lambda h: setattr(_m, "_hook", h)
    _m.get_axon_ntff_profile_hook = lambda: _m._hook
    sys.modules["antenv.axon_hooks"] = _m
    antenv.axon_hooks = _m
    try:
        from trn_agent_boot.trn_boot import _ntff_profile_via_ctypes

        _m._hook = _ntff_profile_via_ctypes("/opt/axon/libaxon_pjrt.so")
    except Exception:
        pass


N, NC, NPC = 50000, 8, 6250
H = 64
H2 = 128
F_IN = 128
LN_EPS = 1e-5
BN_EPS = 1e-5
GEN_EPS = 1e-7

NB = 49          # dst blocks per core (128 nodes, last has 106)
NA_BLK = 25      # group A: blocks 0..24
RA = NA_BLK * 128       # 3200 rows/core in table A
RB = (NB - NA_BLK) * 128  # 3072 rows/core in table B
TABA = NC * RA   # 25600
TABB = NC * RB   # 24576
CHUNK = 4        # dst blocks per gather chunk
NCOLS = NB * 128  # 6272

LAST_EXEC_NS = None


def _preprocess_edges(edge_index):
    """Cross-core-uniform tile geometry + per-core gather index / dst-label
    arrays.

    Returns (nt, gt0, chunks, TOT, MAXNT, idx16, drl) where
      nt[g][b]   tiles for (group g, dst block b)        (uniform)
      gt0[g][b]  global tile index of first tile of (g,b)
      chunks[g]  list of (b_lo, b_hi, ct0, ctiles) per gather chunk
      idx16      [NC, 128, TOT*8] int16 gather indices (16-wrapped, replicated)
      drl        [NC, 128, TOT] f16 dst position labels (255 = pad)
    """
    src = np.asarray(edge_index[0], dtype=np.int64)
    dst = np.asarray(edge_index[1], dtype=np.int64)
    c = dst // NPC
    ld = dst - c * NPC
    db = ld >> 7
    dp = ld & 127
    cs = src // NPC
    ls = src - cs * NPC
    bs = ls >> 7
    ps = ls & 127
    grp = (bs >= NA_BLK).astype(np.int64)
    row = np.where(
        grp == 0,
        cs * RA + ps * NA_BLK + bs,
        cs * RB + ps * (NB - NA_BLK) + (bs - NA_BLK),
    ).astype(np.int64)

    # per (core, grp, block) edge counts -> uniform tile counts
    key = (c * 2 + grp) * NB + db
    cnt = np.bincount(key, minlength=NC * 2 * NB).reshape(NC, 2, NB)
    nt = np.maximum(1, -(-cnt.max(axis=0) // 128))  # [2, NB] cross-core max

    # chunk partition of blocks (same for both groups)
    blos = list(range(0, NB, CHUNK))
    chunk_ranges = [(b0, min(b0 + CHUNK, NB)) for b0 in blos]

    # global tile numbering: group 0 chunks then group 1 chunks
    gt0 = np.zeros((2, NB), dtype=np.int64)
    chunks = [[], []]
    t = 0
    for g in range(2):
        for (b0, b1) in chunk_ranges:
            ct0 = t
            for b in range(b0, b1):
                gt0[g, b] = t
                t += int(nt[g, b])
            chunks[g].append((b0, b1, ct0, t - ct0))
    TOT = t
    MAXNT = int(nt.max())

    # per-edge flat slot: gt0[g,db]*128 + rank within (c,g,db)
    order = np.lexsort((db, grp, c))
    inv = np.empty_like(order)
    inv[order] = np.arange(len(order))
    # rank within each (c,g,db) bucket
    sorted_key = key[order]
    starts = np.searchsorted(sorted_key, np.arange(NC * 2 * NB), side="left")
    rank_sorted = np.arange(len(order)) - starts[sorted_key]
    rank = np.empty_like(rank_sorted)
    rank[order] = rank_sorted

    slot = gt0[grp, db] * 128 + rank  # within-core flat element slot

    idx16 = np.zeros((NC, 16, TOT * 8), dtype=np.int16)
    drl = np.full((NC, 128, TOT), 255.0, dtype=np.float16)
    for cc in range(NC):
        m = c == cc
        fl_idx = np.zeros(TOT * 128, dtype=np.int16)
        fl_drl = np.full(TOT * 128, 255, dtype=np.int64)
        fl_idx[slot[m]] = row[m].astype(np.int16)
        fl_drl[slot[m]] = dp[m]
        # idx wrap: element i -> [i%16, i//16]
        idx16[cc] = fl_idx.reshape(TOT * 8, 16).T
        # drl: element i of tile t -> [i%128, t]
        drl[cc] = fl_drl.reshape(TOT, 128).T.astype(np.float16)
    idx16 = np.tile(idx16, (1, 8, 1))  # replicate to 128 partitions
    return nt, gt0, chunks, TOT, MAXNT, idx16, drl


def kernel(
    x,
    edge_index,
    enc_W,
    enc_b,
    conv_t,
    conv_W1,
    conv_b1,
    conv_lng,
    conv_lnb,
    conv_W2,
    conv_b2,
    block_lng,
    block_lnb,
    fin_t,
    fin_W1,
    fin_b1,
    fin_bng,
    fin_bnb,
    fin_W2,
    fin_b2,
    _trace=False,
):
    global LAST_EXEC_NS
    _install_bir_patch()
    if _trace:
        _install_trace_hook()

    import concourse.bass as bass
    import concourse.mybir as mybir
    import concourse.tile as tile
    from concourse import library_config
    from concourse.bass_utils import run_bass_kernel_spmd
    from concourse.library_overlay import lower_extended_insts
    f32 = mybir.dt.float32
    f16 = mybir.dt.float16
    i16 = mybir.dt.int16
    AF = mybir.ActivationFunctionType
    OP = mybir.AluOpType
    AX = mybir.AxisListType

    x = np.asarray(x, dtype=np.float32)
    nt, gt0, chunks, TOT, MAXNT, idx16, drl = _preprocess_edges(
        np.asarray(edge_index)
    )
    NCH = len(chunks[0])  # chunks per group

    # ---------------- host-side parameter prep (replicated) ----------------
    rep = lambda v, w: np.ascontiguousarray(
        np.broadcast_to(np.asarray(v, np.float32).reshape(1, w), (128, w))
    )
    g_fin = np.asarray(fin_bng, np.float32) / np.sqrt(np.float32(1.0 + BN_EPS))
    # fold the eval-mode BN scale into fin_W1 (exact: per-column scale)
    fin_W1_eff = np.asarray(fin_W1, np.float32) * g_fin[None, :]
    w1all = np.concatenate(
        [np.asarray(conv_W1, np.float32), fin_W1_eff[None]], 0
    ).astype(np.float16)  # [5, 64, 128]
    w2all = np.concatenate(
        [np.asarray(conv_W2, np.float32), np.asarray(fin_W2, np.float32)[None]], 0
    ).astype(np.float16)  # [5, 128, 64]

    # identity-parameter detection (host-side specialization)
    b1 = np.asarray(conv_b1, np.float32)
    b1c_list = [b1[i] - b1[i].mean() for i in range(4)]
    has_b1c = any(np.any(v != 0) for v in b1c_list)
    bar_fin = np.asarray(fin_b1, np.float32) * g_fin + np.asarray(fin_bnb, np.float32)
    has_bar4 = bool(np.any(bar_fin != 0))
    has_gar = bool(np.any(np.asarray(conv_lng, np.float32) != 1.0))
    has_bar = bool(np.any(np.asarray(conv_lnb, np.float32) != 0.0))
    has_b2 = bool(
        np.any(np.asarray(conv_b2, np.float32) != 0.0)
        or np.any(np.asarray(fin_b2, np.float32) != 0.0)
    )
    has_blk = bool(
        np.any(np.asarray(block_lng, np.float32) != 1.0)
        or np.any(np.asarray(block_lnb, np.float32) != 0.0)
    )
    has_encb = bool(np.any(np.asarray(enc_b, np.float32) != 0.0))

    b1c = np.concatenate(
        [rep(v, H2) for v in b1c_list + [np.zeros(H2, np.float32)]], axis=1
    )  # [128, 5*128]
    garr = np.concatenate(
        [rep(v, H2) for v in list(np.asarray(conv_lng, np.float32))]
        + [rep(np.ones(H2, np.float32), H2)],
        axis=1,
    )
    barr = np.concatenate(
        [rep(v, H2) for v in list(np.asarray(conv_lnb, np.float32)) + [bar_fin]],
        axis=1,
    )
    b2r = np.concatenate(
        [rep(v, H) for v in list(np.asarray(conv_b2, np.float32)) + [np.asarray(fin_b2)]],
        axis=1,
    )  # [128, 5*64]
    blg = np.asarray(block_lng, np.float32)
    blb = np.asarray(block_lnb, np.float32)
    blkg = np.concatenate([rep(blg[i], H) for i in (1, 2, 3, 0)], axis=1)
    blkb = np.concatenate([rep(blb[i], H) for i in (1, 2, 3, 0)], axis=1)
    tvals = np.array(
        list(np.asarray(conv_t, np.float32)) + [float(np.asarray(fin_t))], np.float32
    )
    tsc = np.ascontiguousarray(np.broadcast_to(tvals.reshape(1, 5), (128, 5)))
    tbi = np.ascontiguousarray(tsc * np.float32(GEN_EPS))
    iota_rep = np.tile(np.arange(128, dtype=np.float16), (128, 1))  # [128,128]
    encW = np.asarray(enc_W, np.float32).astype(np.float16)  # [128, 64]
    encb = rep(enc_b, H)

    # per-core transposed x: xT[c][f, b*128+p] = x[c*NPC + b*128 + p, f]
    xT = np.zeros((NC, 128, NCOLS), dtype=np.float16)
    for cc in range(NC):
        xc = x[cc * NPC : (cc + 1) * NPC]  # [6250, 128]
        full = np.zeros((NCOLS, 128), np.float32)
        full[: xc.shape[0]] = xc
        xT[cc] = full.T.astype(np.float16)

    # ---------------- build the Bass program ----------------
    nc = bass.Bass(num_swdge_queues=4, dynamic_dma_scratch_size=32768)

    d_xT = nc.dram_tensor("xT", [128, NCOLS], f16, kind="ExternalInput")
    d_idx = nc.dram_tensor("idx16", [128, TOT * 8], i16, kind="ExternalInput")
    d_drl = nc.dram_tensor("drl", [128, TOT], f16, kind="ExternalInput")
    d_w1 = nc.dram_tensor("w1all", [5, H, H2], f16, kind="ExternalInput")
    d_w2 = nc.dram_tensor("w2all", [5, H2, H], f16, kind="ExternalInput")
    if has_b1c:
        d_b1c = nc.dram_tensor("b1c", [128, 5 * H2], f32, kind="ExternalInput")
    if has_gar:
        d_gar = nc.dram_tensor("garr", [128, 5 * H2], f32, kind="ExternalInput")
    if has_bar or has_bar4:
        d_bar = nc.dram_tensor("barr", [128, 5 * H2], f32, kind="ExternalInput")
    if has_b2:
        d_b2r = nc.dram_tensor("b2r", [128, 5 * H], f32, kind="ExternalInput")
    if has_blk:
        d_blkg = nc.dram_tensor("blkg", [128, 4 * H], f32, kind="ExternalInput")
        d_blkb = nc.dram_tensor("blkb", [128, 4 * H], f32, kind="ExternalInput")
    d_tsc = nc.dram_tensor("tsc", [128, 5], f32, kind="ExternalInput")
    d_tbi = nc.dram_tensor("tbi", [128, 5], f32, kind="ExternalInput")
    d_lneps = nc.dram_tensor("lneps", [128, 1], f32, kind="ExternalInput")
    d_iota = nc.dram_tensor("iota", [128, 128], f16, kind="ExternalInput")
    d_ident = nc.dram_tensor("ident", [128, 128], f16, kind="ExternalInput")
    d_encW = nc.dram_tensor("encW", [128, H], f16, kind="ExternalInput")
    if has_encb:
        d_encb = nc.dram_tensor("encb", [128, H], f32, kind="ExternalInput")
    d_out = nc.dram_tensor("out", [NCOLS, H], f32, kind="ExternalOutput")

    d_TinA = nc.dram_tensor("T_in_a", [128, RA], f16)
    d_TinB = nc.dram_tensor("T_in_b", [128, RB], f16)
    d_tabA = [
        nc.dram_tensor(f"T_tabA{i}", [TABA, H2], f16, addr_space="Shared")
        for i in range(2)
    ]
    d_tabB = [
        nc.dram_tensor(f"T_tabB{i}", [TABB, H2], f16, addr_space="Shared")
        for i in range(2)
    ]
    d_tabs = [d_tabA, d_tabB]

    NBH = NB * H  # 3136

    # max tiles in one gather chunk
    TPG = max(int(ct) for g in range(2) for (_, _, _, ct) in chunks[g])
    GMAX = int(os.environ.get("GMAX_TILES", "8"))

    # PSUM bank groups of up to 4 dst blocks, split at the A/B boundary
    bank_groups = []
    for lo in range(0, NA_BLK, 4):
        bank_groups.append((lo, min(lo + 4, NA_BLK)))
    for lo in range(NA_BLK, NB, 4):
        bank_groups.append((lo, min(lo + 4, NB)))
    b2grp = {}
    for (lo, hi) in bank_groups:
        for b in range(lo, hi):
            b2grp[b] = (lo, hi)

    with tile.TileContext(nc) as tc:
        nc.gpsimd.load_library(library_config.mlp)
        nidx_reg = nc.gpsimd.alloc_register(name="nidx")
        with (
            tc.tile_pool(name="state", bufs=1) as st,
            tc.tile_pool(name="wkw", bufs=1) as wk,
            tc.tile_pool(name="wks", bufs=2) as ws,
            tc.tile_pool(name="wide", bufs=1) as wd,
            tc.tile_pool(name="ga", bufs=4) as gpa,
            tc.tile_pool(name="gb", bufs=2) as gpb,
            tc.tile_pool(name="ohp", bufs=1) as ohp,
            tc.tile_pool(name="psa", bufs=3, space="PSUM") as pp,
            tc.tile_pool(name="psq", bufs=1, space="PSUM") as pq,
        ):
            # ---------------- persistent state / constants ----------------
            idx_sb = st.tile([128, TOT * 8], i16, tag="idx")
            nc.sync.dma_start(out=idx_sb[:], in_=d_idx[:])
            drl_sb = st.tile([128, TOT], f16, tag="drl")
            nc.sync.dma_start(out=drl_sb[:], in_=d_drl[:])
            iota_sb = st.tile([128, 128], f16, tag="iota")
            nc.sync.dma_start(out=iota_sb[:], in_=d_iota[:])
            ident = st.tile([128, 128], f16, tag="ident")
            nc.sync.dma_start(out=ident[:], in_=d_ident[:])
            if has_b1c:
                b1c_sb = st.tile([128, 5 * H2], f32, tag="b1c")
                nc.sync.dma_start(out=b1c_sb[:], in_=d_b1c[:])
            if has_gar:
                gar_sb = st.tile([128, 5 * H2], f32, tag="gar")
                nc.sync.dma_start(out=gar_sb[:], in_=d_gar[:])
            if has_bar or has_bar4:
                bar_sb = st.tile([128, 5 * H2], f32, tag="bar")
                nc.sync.dma_start(out=bar_sb[:], in_=d_bar[:])
            if has_b2:
                b2r_sb = st.tile([128, 5 * H], f32, tag="b2r")
                nc.sync.dma_start(out=b2r_sb[:], in_=d_b2r[:])
            if has_blk:
                blkg_sb = st.tile([128, 4 * H], f32, tag="blkg")
                nc.sync.dma_start(out=blkg_sb[:], in_=d_blkg[:])
                blkb_sb = st.tile([128, 4 * H], f32, tag="blkb")
                nc.sync.dma_start(out=blkb_sb[:], in_=d_blkb[:])
            tsc_sb = st.tile([128, 5], f32, tag="tsc")
            nc.sync.dma_start(out=tsc_sb[:], in_=d_tsc[:])
            tbi_sb = st.tile([128, 5], f32, tag="tbi")
            nc.sync.dma_start(out=tbi_sb[:], in_=d_tbi[:])
            lneps_sb = st.tile([128, 1], f32, tag="lneps")
            nc.sync.dma_start(out=lneps_sb[:], in_=d_lneps[:])
            if has_encb:
                encb_sb = st.tile([128, H], f32, tag="encb")
                nc.sync.dma_start(out=encb_sb[:], in_=d_encb[:])
            encW_sb = st.tile([128, H], f16, tag="encW")
            nc.sync.dma_start(out=encW_sb[:], in_=d_encW[:])
            xT_f32 = wd.tile([128, NBH], f32, tag="w64")
            xT_sb = xT_f32[:].bitcast(f16)
            nc.sync.dma_start(out=xT_sb, in_=d_xT[:])
            tlocA = st.tile([128, RA], f16, tag="tlocA")
            tlocB = st.tile([128, RB], f16, tag="tlocB")

            h_a = st.tile([128, NBH], f32, tag="h_a")
            h_b = st.tile([128, NBH], f32, tag="h_b")
            r_sb = st.tile([128, NBH], f16, tag="r_sb")

            h_cur, h_nxt = h_a, h_b

            def t_chunk_group(g, lidx, initial, defer=False):
                """Compute [P2|P1] rows for group g's blocks into tloc{A,B},
                then DMA to the contribution buffer + AllGather into the
                parity table for layer lidx."""
                b0 = 0 if g == 0 else NA_BLK
                b1 = NA_BLK if g == 0 else NB
                nb = b1 - b0
                n64 = nb * H
                sl = slice(b0 * H, b1 * H)
                tloc = tlocA if g == 0 else tlocB
                tv = tloc[:].rearrange("p (b f) -> p b f", f=H2)
                if initial:
                    tm = wk.tile([128, NA_BLK * H], f16, tag="uu")
                    nc.scalar.activation(
                        out=tm[:, :n64], in_=r_sb[:, sl], func=AF.Relu
                    )
                    tm_ap = tm[:, :n64]
                else:
                    tm_ap = r_sb[:, sl]
                tm3 = tm_ap.rearrange("p (b f) -> p b f", f=H)
                # P1 = exp(t*msg) written straight into the right half
                nc.scalar.activation(
                    out=tv[:, :, H:H2],
                    in_=tm3,
                    func=AF.Exp,
                    bias=tbi_sb[:, lidx : lidx + 1],
                    scale=tsc_sb[:, lidx : lidx + 1],
                )
                # P2 = (msg + eps) * P1 in one DVE op, into the left half
                nc.vector.scalar_tensor_tensor(
                    out=tv[:, :, 0:H],
                    in0=tm3,
                    scalar=GEN_EPS,
                    in1=tv[:, :, H:H2],
                    op0=OP.add,
                    op1=OP.mult,
                )
                d_tin = d_TinA if g == 0 else d_TinB
                nc.sync.dma_start(out=d_tin[:], in_=tloc[:])

                def emit_ag():
                    nc.gpsimd.collective_compute(
                        "AllGather",
                        OP.bypass,
                        replica_groups=[list(range(NC))],
                        ins=[d_tin[:]],
                        outs=[d_tabs[g][lidx % 2][:]],
                    )

                if defer:
                    return emit_ag
                emit_ag()

            def node_phase(g, l, wide64, b0, b1):
                """MLP + residual + LN64 for blocks [b0, b1) of group g."""
                nb = b1 - b0
                n64 = nb * H
                n128 = nb * H2
                sl64 = slice(b0 * H, b1 * H)
                l2 = slice(l * H2, (l + 1) * H2)
                lh = slice(l * H, (l + 1) * H)

                # u = agg + r  (f16)
                uu = wk.tile([128, NA_BLK * H], f16, tag="uu")
                nc.vector.tensor_tensor(
                    out=uu[:, :n64], in0=wide64[:, sl64], in1=r_sb[:, sl64], op=OP.add
                )
                # per-block transpose + W1 matmul
                h1w = wk.tile([128, NA_BLK * H2], f16, tag="h1w")
                for i in range(nb):
                    ps_t = pq.tile([H, 128], f16, tag="ptr")
                    nc.tensor.transpose(
                        out=ps_t[:], in_=uu[:, i * H : (i + 1) * H], identity=ident[:]
                    )
                    uT = ws.tile([H, 128], f16, tag="uT")
                    nc.scalar.copy(out=uT[:], in_=ps_t[:])
                    ps1 = pq.tile([128, H2], f32, tag="ph1")
                    nc.tensor.matmul(
                        out=ps1[:], lhsT=uT[:], rhs=w1_sb[:], start=True, stop=True
                    )
                    nc.scalar.copy(out=h1w[:, i * H2 : (i + 1) * H2], in_=ps1[:])
                h1v = h1w[:, :n128].rearrange("p (b f) -> p b f", f=H2)
                if l < 4:
                    # LayerNorm over 128 features, all blocks at once
                    s1 = wk.tile([128, NA_BLK], f32, tag="s1")
                    nc.vector.reduce_sum(out=s1[:, :nb], in_=h1v, axis=AX.X)
                    nmu = wk.tile([128, NA_BLK], f32, tag="nmu")
                    nc.vector.tensor_scalar_mul(
                        out=nmu[:, :nb], in0=s1[:, :nb], scalar1=-1.0 / H2
                    )
                    hc = wk.tile([128, NA_BLK * H2], f16, tag="hc")
                    hcv = hc[:, :n128].rearrange("p (b f) -> p b f", f=H2)
                    nc.vector.tensor_tensor(
                        out=hcv,
                        in0=h1v,
                        in1=nmu[:, :nb].unsqueeze(2).to_broadcast([128, nb, H2]),
                        op=OP.add,
                    )
                    if has_b1c:
                        nc.vector.tensor_tensor(
                            out=hcv,
                            in0=hcv,
                            in1=b1c_sb[:, l2].unsqueeze(1).to_broadcast([128, nb, H2]),
                            op=OP.add,
                        )
                    sq = wk.tile([128, NA_BLK * H2], f16, tag="sq")
                    nc.scalar.activation(
                        out=sq[:, :n128], in_=hc[:, :n128], func=AF.Square
                    )
                    s2 = wk.tile([128, NA_BLK], f32, tag="s2")
                    nc.vector.reduce_sum(
                        out=s2[:, :nb],
                        in_=sq[:, :n128].rearrange("p (b f) -> p b f", f=H2),
                        axis=AX.X,
                    )
                    sd = wk.tile([128, NA_BLK], f32, tag="sd")
                    nc.scalar.activation(
                        out=sd[:, :nb], in_=s2[:, :nb], func=AF.Sqrt,
                        bias=lneps_sb[:], scale=1.0 / H2,
                    )
                    rstd = wk.tile([128, NA_BLK], f32, tag="rstd")
                    nc.vector.reciprocal(out=rstd[:, :nb], in_=sd[:, :nb])
                    hn = wk.tile([128, NA_BLK * H2], f16, tag="hn")
                    hnv = hn[:, :n128].rearrange("p (b f) -> p b f", f=H2)
                    nc.vector.tensor_tensor(
                        out=hnv,
                        in0=hcv,
                        in1=rstd[:, :nb].unsqueeze(2).to_broadcast([128, nb, H2]),
                        op=OP.mult,
                    )
                    if has_gar:
                        nc.vector.tensor_tensor(
                            out=hnv,
                            in0=hnv,
                            in1=gar_sb[:, l2].unsqueeze(1).to_broadcast([128, nb, H2]),
                            op=OP.mult,
                        )
                    if has_bar:
                        nc.vector.tensor_tensor(
                            out=hnv,
                            in0=hnv,
                            in1=bar_sb[:, l2].unsqueeze(1).to_broadcast([128, nb, H2]),
                            op=OP.add,
                        )
                    hn_ap = hn
                else:
                    if has_bar4:
                        nc.vector.tensor_tensor(
                            out=h1v,
                            in0=h1v,
                            in1=bar_sb[:, l2].unsqueeze(1).to_broadcast([128, nb, H2]),
                            op=OP.add,
                        )
                    hn_ap = h1w
                # relu on the Scalar engine
                r1 = wk.tile([128, NA_BLK * H2], f16, tag="hc" if l >= 4 else "sq")
                nc.scalar.activation(
                    out=r1[:, :n128], in_=hn_ap[:, :n128], func=AF.Relu
                )
                # per-block transpose + W2 matmul -> h_nxt / wide64
                for i in range(nb):
                    ps_t2 = pq.tile([128, 128], f16, tag="ptr2")
                    nc.tensor.transpose(
                        out=ps_t2[:], in_=r1[:, i * H2 : (i + 1) * H2],
                        identity=ident[:],
                    )
                    r1T = ws.tile([128, 128], f16, tag="r1T")
                    nc.scalar.copy(out=r1T[:], in_=ps_t2[:])
                    ps2 = pq.tile([128, H], f32, tag="ph2")
                    nc.tensor.matmul(
                        out=ps2[:], lhsT=r1T[:], rhs=w2_sb[:], start=True, stop=True
                    )
                    if l == 0 or l == 4:
                        nc.scalar.copy(
                            out=h_nxt[:, (b0 + i) * H : (b0 + i + 1) * H], in_=ps2[:]
                        )
                    else:
                        nc.scalar.copy(
                            out=wide64[:, (b0 + i) * H : (b0 + i + 1) * H], in_=ps2[:]
                        )
                b2b = None
                if has_b2:
                    b2b = b2r_sb[:, lh].unsqueeze(1).to_broadcast([128, nb, H])
                if l == 0 or l == 4:
                    if has_b2:
                        hx = h_nxt[:, sl64].rearrange("p (b f) -> p b f", f=H)
                        nc.vector.tensor_tensor(out=hx, in0=hx, in1=b2b, op=OP.add)
                    if l == 4:
                        return
                else:
                    if has_b2:
                        co_v = wide64[:, sl64].rearrange("p (b f) -> p b f", f=H)
                        nc.vector.tensor_tensor(
                            out=co_v, in0=co_v, in1=b2b, op=OP.add
                        )
                    nc.vector.tensor_tensor(
                        out=h_nxt[:, sl64], in0=wide64[:, sl64], in1=h_cur[:, sl64],
                        op=OP.add,
                    )
                # LN64 (block norm for next conv) + relu -> r_sb
                hv = h_nxt[:, sl64].rearrange("p (b f) -> p b f", f=H)
                s1b = wk.tile([128, NA_BLK], f32, tag="s1b")
                nc.vector.reduce_sum(out=s1b[:, :nb], in_=hv, axis=AX.X)
                nmub = wk.tile([128, NA_BLK], f32, tag="nmub")
                nc.vector.tensor_scalar_mul(
                    out=nmub[:, :nb], in0=s1b[:, :nb], scalar1=-1.0 / H
                )
                hcb = wk.tile([128, NA_BLK * H], f32, tag="hc")
                hcbv = hcb[:, :n64].rearrange("p (b f) -> p b f", f=H)
                nc.vector.tensor_tensor(
                    out=hcbv,
                    in0=hv,
                    in1=nmub[:, :nb].unsqueeze(2).to_broadcast([128, nb, H]),
                    op=OP.add,
                )
                sqb = wk.tile([128, NA_BLK * H], f16, tag="sq")
                nc.scalar.activation(
                    out=sqb[:, :n64], in_=hcb[:, :n64], func=AF.Square
                )
                s2b = wk.tile([128, NA_BLK], f32, tag="s2b")
                nc.vector.reduce_sum(
                    out=s2b[:, :nb],
                    in_=sqb[:, :n64].rearrange("p (b f) -> p b f", f=H),
                    axis=AX.X,
                )
                sdb = wk.tile([128, NA_BLK], f32, tag="sdb")
                nc.scalar.activation(
                    out=sdb[:, :nb], in_=s2b[:, :nb], func=AF.Sqrt,
                    bias=lneps_sb[:], scale=1.0 / H,
                )
                rstdb = wk.tile([128, NA_BLK], f32, tag="rstdb")
                nc.vector.reciprocal(out=rstdb[:, :nb], in_=sdb[:, :nb])
                gsl = slice(l * H, (l + 1) * H)
                hnb = wk.tile([128, NA_BLK * H], f32, tag="h1w")
                hnbv = hnb[:, :n64].rearrange("p (b f) -> p b f", f=H)
                nc.vector.tensor_tensor(
                    out=hnbv,
                    in0=hcbv,
                    in1=rstdb[:, :nb].unsqueeze(2).to_broadcast([128, nb, H]),
                    op=OP.mult,
                )
                if has_blk:
                    nc.vector.tensor_tensor(
                        out=hnbv,
                        in0=hnbv,
                        in1=blkg_sb[:, gsl].unsqueeze(1).to_broadcast([128, nb, H]),
                        op=OP.mult,
                    )
                    nc.vector.tensor_tensor(
                        out=hnbv,
                        in0=hnbv,
                        in1=blkb_sb[:, gsl].unsqueeze(1).to_broadcast([128, nb, H]),
                        op=OP.add,
                    )
                nc.scalar.activation(
                    out=r_sb[:, sl64], in_=hnb[:, :n64], func=AF.Relu
                )

            # ---------------- encoder: r = x @ enc_W + enc_b ----------------
            def enc_block(b):
                ps_e = pq.tile([128, H], f32, tag="pe")
                nc.tensor.matmul(
                    out=ps_e[:],
                    lhsT=xT_sb[:, b * 128 : (b + 1) * 128],
                    rhs=encW_sb[:],
                    start=True,
                    stop=True,
                )
                nc.scalar.copy(out=r_sb[:, b * H : (b + 1) * H], in_=ps_e[:])

            for b in range(NA_BLK):
                enc_block(b)
            if has_encb:
                nc.vector.tensor_tensor(
                    out=r_sb[:, : NA_BLK * H].rearrange("p (b f) -> p b f", f=H),
                    in0=r_sb[:, : NA_BLK * H].rearrange("p (b f) -> p b f", f=H),
                    in1=encb_sb[:].unsqueeze(1).to_broadcast([128, NA_BLK, H]),
                    op=OP.add,
                )
            t_chunk_group(0, 0, initial=True)
            for b in range(NA_BLK, NB):
                enc_block(b)
            if has_encb:
                nbB = NB - NA_BLK
                nc.vector.tensor_tensor(
                    out=r_sb[:, NA_BLK * H :].rearrange("p (b f) -> p b f", f=H),
                    in0=r_sb[:, NA_BLK * H :].rearrange("p (b f) -> p b f", f=H),
                    in1=encb_sb[:].unsqueeze(1).to_broadcast([128, nbB, H]),
                    op=OP.add,
                )
            t_chunk_group(1, 0, initial=True)
            pending_a_ag = None

            NLAYERS = 5
            for l in range(NLAYERS):
                w1_sb = ws.tile([H, H2], f16, tag="w1")
                nc.sync.dma_start(out=w1_sb[:], in_=d_w1[l])
                w2_sb = ws.tile([H2, H], f16, tag="w2")
                nc.sync.dma_start(out=w2_sb[:], in_=d_w2[l])

                par = l % 2
                gbufs = {}
                ohs = {}
                qstate = [0]

                def issue_gather(g, ci):
                    (b0_, b1_, ct0, ctn) = chunks[g][ci]
                    pool = gpa if g == 0 else gpb
                    gt = pool.tile([128, TPG * H2], f16, tag=f"g{g}")
                    for s0 in range(0, ctn, GMAX):
                        sn = min(GMAX, ctn - s0)
                        nc.gpsimd.reg_mov(nidx_reg, sn * 128)
                        nc.gpsimd.dma_gather(
                            gt[:, s0 * H2 : (s0 + sn) * H2].rearrange(
                                "p (t f) -> p t f", f=H2
                            ),
                            d_tabs[g][par][:],
                            idx_sb[:, (ct0 + s0) * 8 : (ct0 + s0 + sn) * 8],
                            sn * 128,
                            nidx_reg,
                            H2,
                            queue_num=qstate[0],
                        )
                        qstate[0] = (qstate[0] + 1) % 4
                    gbufs[(g, ci)] = (gt, ct0)

                def build_oh(ci):
                    for g in range(2):
                        (b0_, b1_, ct0, ctn) = chunks[g][ci]
                        oh = ohp.tile([128, TPG * 128], f16, tag=f"oh{g}")
                        nc.vector.tensor_tensor(
                            out=oh[:, : ctn * 128].rearrange(
                                "p (t f) -> p t f", f=128
                            ),
                            in0=iota_sb[:]
                            .unsqueeze(1)
                            .to_broadcast([128, ctn, 128]),
                            in1=drl_sb[:, ct0 : ct0 + ctn]
                            .unsqueeze(2)
                            .to_broadcast([128, ctn, 128]),
                            op=OP.is_equal,
                        )
                        ohs[(g, ci)] = (oh, ct0)

                issue_gather(0, 0)
                issue_gather(0, 1)
                issue_gather(0, 2)
                issue_gather(0, 3)
                issue_gather(1, 0)
                issue_gather(1, 1)
                build_oh(0)

                wide64 = wd.tile([128, NBH], f32, tag="w64")

                for ci in range(NCH):
                    if ci == 7 and pending_a_ag is not None:
                        pending_a_ag()
                        pending_a_ag = None
                    if ci + 1 < NCH:
                        build_oh(ci + 1)
                    (cb0, cb1, _, _) = chunks[0][ci]
                    for b in range(cb0, cb1):
                        lo, hi = b2grp[b]
                        if b == lo:
                            psw = pp.tile([128, 4 * H2], f32, tag="pagg")
                            psw_cur = psw
                        j = b - lo
                        out_ap = psw_cur[:, j * H2 : (j + 1) * H2]
                        tot_tiles = int(nt[0][b]) + int(nt[1][b])
                        done = 0
                        for g in range(2):
                            oh, oct0 = ohs[(g, ci)]
                            gt, gct0 = gbufs[(g, ci)]
                            t0 = int(gt0[g][b])
                            ntb = int(nt[g][b])
                            for t in range(ntb):
                                done += 1
                                nc.tensor.matmul(
                                    out=out_ap,
                                    lhsT=oh[
                                        :,
                                        (t0 - oct0 + t) * 128 : (t0 - oct0 + t + 1)
                                        * 128,
                                    ],
                                    rhs=gt[
                                        :,
                                        (t0 - gct0 + t) * H2 : (t0 - gct0 + t + 1)
                                        * H2,
                                    ],
                                    start=(done == 1),
                                    stop=(done == tot_tiles),
                                )
                        if b == hi - 1:
                            # softmax epilogue for the whole bank
                            k = hi - lo
                            v = psw_cur[:, : k * H2].rearrange(
                                "p (b f) -> p b f", f=H2
                            )
                            dmx = ws.tile([128, 4 * H], f32, tag="dmx")
                            dv = dmx[:, : k * H].rearrange("p (b f) -> p b f", f=H)
                            nc.vector.tensor_scalar_max(
                                out=dv, in0=v[:, :, H:H2], scalar1=1e-16
                            )
                            rec = ws.tile([128, 4 * H], f32, tag="rec")
                            rv = rec[:, : k * H].rearrange("p (b f) -> p b f", f=H)
                            nc.vector.reciprocal(out=rv, in_=dv)
                            nc.vector.tensor_tensor(
                                out=wide64[:, lo * H : hi * H].rearrange(
                                    "p (b f) -> p b f", f=H
                                ),
                                in0=v[:, :, 0:H],
                                in1=rv,
                                op=OP.mult,
                            )
                        if b == 11:
                            node_phase(0, l, wide64, 0, 12)
                        elif b == 23:
                            node_phase(0, l, wide64, 12, 24)
                        elif b == NA_BLK - 1:
                            node_phase(0, l, wide64, 24, NA_BLK)
                            if l < 4:
                                pending_a_ag = t_chunk_group(
                                    0, l + 1, initial=False, defer=True
                                )
                        elif b == 36:
                            node_phase(1, l, wide64, NA_BLK, 37)
                        elif b == 44:
                            node_phase(1, l, wide64, 37, 45)
                    if ci + 4 < NCH:
                        issue_gather(0, ci + 4)
                    if ci + 2 < NCH:
                        issue_gather(1, ci + 2)
                node_phase(1, l, wide64, 45, NB)
                if l < 4:
                    t_chunk_group(1, l + 1, initial=False)
                    h_cur, h_nxt = h_nxt, h_cur

            # h_nxt (not swapped after l=4) holds the final output
            nc.sync.dma_start(
                out=d_out[:].rearrange("(b p) f -> p b f", p=128),
                in_=h_nxt[:].rearrange("p (b f) -> p b f", f=H),
            )

    lower_extended_insts(nc)

    in_maps = []
    for cc in range(NC):
        m = {
            "xT": np.ascontiguousarray(xT[cc]),
            "idx16": np.ascontiguousarray(idx16[cc]),
            "drl": np.ascontiguousarray(drl[cc]),
            "w1all": w1all,
            "w2all": w2all,
            "tsc": tsc,
            "tbi": tbi,
            "lneps": np.full((128, 1), LN_EPS, np.float32),
            "iota": iota_rep,
            "ident": np.eye(128, dtype=np.float16),
            "encW": encW,
        }
        if has_b1c:
            m["b1c"] = b1c
        if has_gar:
            m["garr"] = garr
        if has_bar or has_bar4:
            m["barr"] = barr
        if has_b2:
            m["b2r"] = b2r
        if has_blk:
            m["blkg"] = blkg
            m["blkb"] = blkb
        if has_encb:
            m["encb"] = encb
        in_maps.append(m)
    res = run_bass_kernel_spmd(nc, in_maps, list(range(NC)), trace=_trace)
    LAST_EXEC_NS = res.exec_time_ns
    out = np.empty((N, H), dtype=np.float32)
    for cc in range(NC):
        oc = res.results[cc]["out"]
        out[cc * NPC : (cc + 1) * NPC] = oc[:NPC]
    return out.astype(np.float32)
